# revision 16
# baseline (speedup 1.0000x reference)
"""AdaptiveBlock Trainium2 kernel, 8-core data-parallel.

Reference computation (per batch b):
    y      = mean(x[b], axis=(H, W))                    # (C,)
    h      = gelu(y @ w1.T)                             # (HIDDEN,)
    y'     = gelu(h @ w2.T)                             # (C,)
    A      = (y' @ wA.T).reshape(H, R)
    Bm     = (y' @ wB.T).reshape(R, W)
    attn   = sigmoid(A @ Bm)                            # (H, W)
    out[b] = broadcast attn over C                      # (C, H, W)

Sharding: B=32 split over 8 cores (4 batches/core), weights replicated.
Memory-bound: each core reads 13.9 MB (x + packed weights) and writes
12.8 MB; per-core DMA ceiling is ~427 GB/s (16 engines x 26.7 GB/s for
any packet >= 6 KB), so the byte floor is ~63 us + ~11 us fixed NEFF
preamble.

Schedule (the whole point of this implementation):
- ALL big HBM transfers ride the single Sync-engine HWDGE queue in
  program order: weights, x chunks b0..b3, then the four outputs as
  they become ready.  In-queue FIFO ordering = automatic read priority
  (descriptors for writes sit behind the remaining read descriptors),
  and writes backfill the engines the moment the last read drains --
  no idle gap, no read/write round-robin stretching the read phase.
  The previous revision put reads on the gpsimd SWDGE queue and writes
  on sync; the trace showed reads 12-51 us, first write at 58 us, last
  write trigger at 88 us, dead air at 50-60 and 75-85 us.
- Small SBUF->SBUF moves (bilinear operand extraction, map flatten)
  ride the gpsimd SWDGE queue so they never queue behind megabytes of
  input descriptors.
- Spatial-sum reduces alternate engines per chunk (cc0 -> DVE
  reduce, cc1 -> ACT activation accum_out) so neither engine falls
  behind the ~3.7 us/chunk input cadence; the final chunk (b3 cc1) is
  split in half across both engines to cut the tail latency.
- The per-batch broadcast (attn row -> 128 partitions via K=1
  ones-matmul) copies PSUM->SBUF in 1024-wide pieces (2 PSUM banks,
  ~half the per-element overhead of 512-wide) with a hand-interleaved
  emission order so each engine's FIFO serves the late reduces and the
  earlier batches' copies in ready-time order -- every output trigger
  lands before the write stream would drain dry.
- Tanh + Gelu activation tables are prefetched at t~9 us (ACT idle)
  instead of faulting in mid-kernel on the busiest engine.

Numerics: weights pre-transposed and pre-cast to bf16 host-side
(contraction dim on partitions); PSUM f32.  Activations ~1e-2, output
sigmoid ~0.5, so bf16 noise is ~1e-6 relative on the output.  wA's
output columns are permuted host-side from i*8+r to r*64+i so the
per-batch (8, 56) bilinear lhsT is a contiguous-row SBUF->SBUF DMA.
sigmoid(x) = 0.5 + 0.5*tanh(x/2) is realized as Tanh activation on the
(56, 56) map followed by an affine PSUM->SBUF copy after broadcast.
"""

import numpy as np
import ml_dtypes

import concourse.bass as bass
import concourse.tile as tile
from concourse import bacc, mybir
from concourse.bass_utils import run_bass_kernel_spmd

F32 = mybir.dt.float32
BF16 = mybir.dt.bfloat16

B, C, H, W = 32, 256, 56, 56
HW = H * W                      # 3136
HIDDEN = 512
RANK = 8
HR = H * RANK                   # 448
NCORES = 8
BLOC = B // NCORES              # 4 batches per core
P = 128
NCC = C // P                    # 2 channel chunks
NHH = HIDDEN // P               # 4 hidden chunks
BCHUNK = 512                    # matmul moving free-dim max (PSUM bank)
# batch -> (group, slot): group A = (b0, b1) computed mid-stream; b2 and
# b3 run solo so neither waits on later data -- b3's chain is the only
# one after the read stream ends, and its output needs to be ready a
# full ~12 us before the write stream would drain dry
GROUPS = [[0, 1], [2], [3]]
GMAP = {0: (0, 0), 1: (0, 1), 2: (1, 0), 3: (2, 0)}
NGRP = len(GROUPS)


def build_bass(sim_compat: bool = False) -> bacc.Bacc:
    """sim_compat=True swaps exact Gelu (not implemented in CoreSim) for a
    0.5*x stand-in; with |gelu-input| ~ 0.02 this perturbs the final sigmoid
    output by ~1e-5 relative, so the sim still validates all layout/dataflow.
    Hardware builds always use the exact erf-based Gelu."""
    gelu_f = (
        mybir.ActivationFunctionType.Copy
        if sim_compat
        else mybir.ActivationFunctionType.Gelu
    )
    gelu_s = 0.5 if sim_compat else 1.0
    nc = bacc.Bacc(num_devices=NCORES)

    HRP = 2 * C                                        # 512: r*64+i padded cols
    WPACK = NCC * HIDDEN + NHH * C + 2 * NCC * HRP     # 4096 bf16 columns
    x_d = nc.dram_tensor("x", [BLOC, C, HW], F32, kind="ExternalInput")
    wpk_d = nc.dram_tensor("wpk", [P, WPACK], BF16, kind="ExternalInput")
    out_d = nc.dram_tensor("out", [BLOC, C, HW], F32, kind="ExternalOutput")

    x_v = x_d.ap().rearrange("b (cc p) hw -> b cc p hw", p=P)
    out_v = out_d.ap().rearrange("b (cc p) hw -> b cc p hw", p=P)
    OF_W1 = 0
    OF_W2 = NCC * HIDDEN
    OF_WA = NCC * HIDDEN + NHH * C
    OF_WB = NCC * HIDDEN + NHH * C + NCC * HRP

    with tile.TileContext(nc) as tc:
        with (
            tc.tile_pool(name="xin", bufs=8) as xpool,
            tc.tile_pool(name="persist", bufs=1) as ppool,
            tc.tile_pool(name="small", bufs=2) as spool,
            tc.tile_pool(name="bc", bufs=4) as bcpool,
            tc.tile_pool(name="ps_small", bufs=2, space="PSUM") as ps_small,
            tc.tile_pool(name="ps_ab", bufs=2, space="PSUM") as ps_ab,
        ):
            wpk = ppool.tile([P, WPACK], BF16, tag="wpk", name="wpk")
            w1t = [wpk[:, OF_W1 + cc * HIDDEN : OF_W1 + (cc + 1) * HIDDEN]
                   for cc in range(NCC)]
            w2t = [wpk[:, OF_W2 + hh * C : OF_W2 + (hh + 1) * C]
                   for hh in range(NHH)]
            wat = [wpk[:, OF_WA + cc * HRP : OF_WA + (cc + 1) * HRP]
                   for cc in range(NCC)]
            wbt = [wpk[:, OF_WB + cc * HRP : OF_WB + (cc + 1) * HRP]
                   for cc in range(NCC)]
            # prefetch both activation tables while ACT is idle
            warm = ppool.tile([1, 2], F32, tag="warm", name="warm")
            nc.vector.memset(warm[:], 0.0)
            nc.scalar.activation(warm[:], warm[:],
                                 mybir.ActivationFunctionType.Sigmoid)
            nc.scalar.activation(warm[:], warm[:], gelu_f)

            ysum = [[ppool.tile([P, len(GROUPS[g])], F32,
                                tag=f"ysum{g}{cc}", name=f"ysum{g}{cc}")
                     for cc in range(NCC)] for g in range(NGRP)]
            hparts = ppool.tile([P, 2], F32, tag="hparts", name="hparts")

            def load_chunk(b, cc):
                g, j = GMAP[b]
                xt = xpool.tile([P, HW], F32, tag="xt", name="xt")
                if (b, cc) == (BLOC - 1, NCC - 1):
                    # split the final chunk across both engines: ~2 us
                    # reduce tail after the read stream ends, not ~4
                    HH = HW // 2
                    nc.sync.dma_start(xt[:, 0:HH], x_v[b, cc][:, 0:HH])
                    nc.scalar.activation(
                        xt[:, 0:HH], xt[:, 0:HH],
                        mybir.ActivationFunctionType.Copy,
                        accum_out=hparts[:, 0:1],
                    )
                    nc.sync.dma_start(xt[:, HH:HW], x_v[b, cc][:, HH:HW])
                    nc.vector.reduce_sum(hparts[:, 1:2], xt[:, HH:HW],
                                         axis=mybir.AxisListType.X)
                    nc.vector.tensor_add(
                        ysum[g][cc][:, j : j + 1], hparts[:, 0:1],
                        hparts[:, 1:2],
                    )
                else:
                    nc.sync.dma_start(xt[:], x_v[b, cc])
                    if cc == 0:
                        nc.vector.reduce_sum(
                            ysum[g][cc][:, j : j + 1], xt[:],
                            axis=mybir.AxisListType.X,
                        )
                    else:
                        nc.scalar.activation(
                            xt[:], xt[:], mybir.ActivationFunctionType.Copy,
                            accum_out=ysum[g][cc][:, j : j + 1],
                        )

            def make_ysb(g, eng):
                gb = len(GROUPS[g])
                ysb = [ppool.tile([P, gb], BF16, tag=f"ysb{g}{cc}",
                                  name=f"ysb{g}{cc}") for cc in range(NCC)]
                for cc in range(NCC):
                    if eng == "scalar":
                        nc.scalar.copy(ysb[cc][:], ysum[g][cc][:])
                    else:
                        nc.vector.tensor_copy(ysb[cc][:], ysum[g][cc][:])
                return ysb

            def mlp_group(g, ysb):
                """MLP + per-batch A|B rows for one batch group."""
                gb = len(GROUPS[g])
                hT = [ppool.tile([P, gb], BF16, tag=f"hT{g}{hh}",
                                 name=f"hT{g}{hh}") for hh in range(NHH)]
                for hh in range(NHH):
                    ph = ps_small.tile([P, gb], F32, tag="ps", name="ps")
                    for cc in range(NCC):
                        nc.tensor.matmul(
                            ph[:], w1t[cc][:, hh * P : (hh + 1) * P], ysb[cc][:],
                            start=(cc == 0), stop=(cc == NCC - 1),
                        )
                    nc.scalar.activation(hT[hh][:], ph[:], gelu_f,
                                         scale=gelu_s / HW)
                ypT = [ppool.tile([P, gb], BF16, tag=f"ypT{g}{cc}",
                                  name=f"ypT{g}{cc}") for cc in range(NCC)]
                for cc in range(NCC):
                    pyp = ps_small.tile([P, gb], F32, tag="ps", name="ps")
                    for hh in range(NHH):
                        nc.tensor.matmul(
                            pyp[:], w2t[hh][:, cc * P : (cc + 1) * P], hT[hh][:],
                            start=(hh == 0), stop=(hh == NHH - 1),
                        )
                    nc.scalar.activation(ypT[cc][:], pyp[:], gelu_f, scale=gelu_s)
                # A|B projection per batch (M=1) so each batch's row sits
                # at partition 0: PE operands must base at partition 0/32/64,
                # and the bilinear reads slices of this row directly
                abjs = []
                for j in range(gb):
                    pab = ps_ab.tile([1, 2 * BCHUNK], F32, tag="pab",
                                     name="pab")
                    for half, wt in ((0, wat), (1, wbt)):
                        for cc in range(NCC):
                            nc.tensor.matmul(
                                pab[:, half * BCHUNK : (half + 1) * BCHUNK],
                                ypT[cc][:, j : j + 1], wt[cc][:],
                                start=(cc == 0), stop=(cc == NCC - 1),
                            )
                    abj = ppool.tile([1, 2 * BCHUNK], BF16, tag=f"ab{g}{j}",
                                     name=f"ab{g}{j}")
                    # single-partition copies are free-dim serial: one per
                    # engine so the two batches' copies run concurrently
                    if j == 0:
                        nc.scalar.copy(abj[:], pab[:])
                    else:
                        nc.vector.tensor_copy(abj[:], pab[:])
                    abjs.append(abj)
                return abjs

            def bilinear_flat(abj):
                """attn = sigmoid(A @ B) flattened to (1, HW) f32.

                A @ B is 8 accumulating rank-1 (K=1) matmuls whose operands
                are slices of batch j's ab_sb row itself -- the 64-aligned
                host-side weight permutation puts A[:, r] at cols r*64:+56
                of the first half and B[r, :] at the same cols of the
                second half, so no cross-partition extraction DMA at all
                (SWDGE scatter moves cost 1 us PER PACKET of latency while
                the big stream runs; the old [8, 64] operands were 8
                packets each)."""
                pm = ps_small.tile([H, W], F32, tag="ps", name="ps")
                for r in range(RANK):
                    nc.tensor.matmul(
                        pm[:],
                        abj[0:1, r * 64 : r * 64 + H],
                        abj[0:1, BCHUNK + r * 64 : BCHUNK + r * 64 + W],
                        start=(r == 0), stop=(r == RANK - 1),
                    )
                msb = spool.tile([H, W], F32, tag="msb", name="msb")
                nc.scalar.activation(msb[:], pm[:],
                                     mybir.ActivationFunctionType.Sigmoid)
                flat = spool.tile([1, HW], F32, tag="flat", name="flat")
                # the 56-row flatten dribbles ~100ns+/packet behind the big
                # stream on any queue -- split it across BOTH small queues
                # (scalar HWDGE + gpsimd SWDGE, 28 packets each, in
                # parallel), and keep it off the sync queue where it would
                # sit behind megabytes of output descriptors
                flat_v = flat[0:1, :].rearrange("o (i j) -> o i j", i=H)
                nc.scalar.dma_start(flat_v[:, 0 : H // 2, :],
                                    msb[0 : H // 2, :])
                nc.gpsimd.dma_start(flat_v[:, H // 2 : H, :],
                                    msb[H // 2 : H, :])
                return flat

            # broadcast halves: gpsimd (fully idle otherwise) replicates
            # the attn row to all 128 partitions directly in SBUF -- no
            # ones-matmul, no PSUM, no PSUM->SBUF copies on ACT/DVE.
            # Two halves per batch so the first output DMA can be in
            # flight while the second half still broadcasts.
            HHW = HW // 2            # 1568 -> 6272B output rows, full rate

            def bcast_half(flat, bc, half):
                lo, hi = (0, HHW) if half == 0 else (HHW, HW)
                nc.gpsimd.partition_broadcast(
                    bc[:, lo:hi], flat[0:1, lo:hi]
                )

            def out_dma_half(b, bc, half):
                lo, hi = (0, HHW) if half == 0 else (HHW, HW)
                for cc in range(NCC):
                    nc.sync.dma_start(out_v[b, cc][:, lo:hi], bc[:, lo:hi])

            bctiles = [bcpool.tile([P, HW], F32, tag="bct", name="bct")
                       for _ in range(BLOC)]

            # ---- emission.  The tile scheduler list-schedules each
            # engine's stream by modeled ready time; the hard ordering
            # constraint is the sync queue: all read triggers before any
            # write trigger (in-queue FIFO = read priority + seamless
            # write backfill), wpk mid-stream (still beats group A's MLP).
            load_chunk(0, 0)            # x lands ~13; DVE reduce
            load_chunk(0, 1)            # ~17; ACT
            load_chunk(1, 0)            # ~21; DVE
            load_chunk(1, 1)            # ~25; ACT
            nc.sync.dma_start(wpk[:], wpk_d.ap())        # lands ~27
            ysbA = make_ysb(0, "vector")
            abA = mlp_group(0, ysbA)    # PE+ACT ~28.5-32
            load_chunk(2, 0)            # ~31; DVE
            flat0 = bilinear_flat(abA[0])   # ~32-38 incl flatten dribble
            load_chunk(2, 1)            # ~35; ACT
            bcast_half(flat0, bctiles[0], 0)     # gpsimd ~38-43
            bcast_half(flat0, bctiles[0], 1)
            flat1 = bilinear_flat(abA[1])
            load_chunk(3, 0)            # ~38.5; DVE
            bcast_half(flat1, bctiles[1], 0)
            load_chunk(3, 1)            # ~42.5; split ACT h1 / DVE h2
            bcast_half(flat1, bctiles[1], 1)
            # output triggers strictly after ALL read triggers (above)
            out_dma_half(0, bctiles[0], 0)
            out_dma_half(0, bctiles[0], 1)   # reads end ~43
            out_dma_half(1, bctiles[1], 0)
            out_dma_half(1, bctiles[1], 1)
            ysbB = make_ysb(1, "vector")     # b2 solo: ready ~40
            abB = mlp_group(1, ysbB)
            flat2 = bilinear_flat(abB[0])
            bcast_half(flat2, bctiles[2], 0)
            bcast_half(flat2, bctiles[2], 1)
            out_dma_half(2, bctiles[2], 0)   # ready ~53, needed ~58
            out_dma_half(2, bctiles[2], 1)
            ysbC = make_ysb(2, "vector")     # b3 solo: ready ~46
            abC = mlp_group(2, ysbC)
            flat3 = bilinear_flat(abC[0])
            bcast_half(flat3, bctiles[3], 0)
            bcast_half(flat3, bctiles[3], 1)
            out_dma_half(3, bctiles[3], 0)   # ready ~60, needed ~66
            out_dma_half(3, bctiles[3], 1)

    nc.compile()
    return nc


def _prep_in_maps(x, w1, w2, wA, wB):
    x = np.ascontiguousarray(np.asarray(x, dtype=np.float32))
    w1 = np.asarray(w1, dtype=np.float32)
    w2 = np.asarray(w2, dtype=np.float32)
    wA = np.asarray(wA, dtype=np.float32)
    wB = np.asarray(wB, dtype=np.float32)

    bf = ml_dtypes.bfloat16
    w1t = np.ascontiguousarray(w1.T)                       # (C, HIDDEN)
    w2t = np.ascontiguousarray(w2.T)                       # (HIDDEN, C)
    # permute wA rows i*8+r -> r*64+i (8 zero pad cols per r) and wB rows
    # r*56+j -> r*64+j, then transpose: 64-aligned r-chunks make the
    # per-batch (8, 56) bilinear operand extraction a clean strided DMA
    HRP = 2 * C
    wap = np.zeros((RANK, 64, C), np.float32)
    wap[:, :H, :] = wA.reshape(H, RANK, C).transpose(1, 0, 2)
    wat = np.ascontiguousarray(wap.reshape(HRP, C).T)
    wbp = np.zeros((RANK, 64, C), np.float32)
    wbp[:, :W, :] = wB.reshape(RANK, W, C)
    wbt = np.ascontiguousarray(wbp.reshape(HRP, C).T)

    # pack per-partition: [w1t cc-chunks | w2t hh-chunks | wat | wbt]
    def chunked(m, n):          # (n*128, F) -> (128, n*F), chunk-major cols
        f = m.shape[1]
        return m.reshape(n, P, f).transpose(1, 0, 2).reshape(P, n * f)

    wpk = np.concatenate(
        [chunked(w1t, NCC), chunked(w2t, NHH), chunked(wat, NCC),
         chunked(wbt, NCC)], axis=1,
    ).astype(bf)

    xs = x.reshape(NCORES, BLOC, C, HW)
    return [{"x": xs[i], "wpk": wpk} for i in range(NCORES)]


_NC_CACHE = None


def _get_nc():
    global _NC_CACHE
    if _NC_CACHE is None:
        _NC_CACHE = build_bass()
    return _NC_CACHE


def run(inputs: dict, trace: bool = False):
    """Run on 8 NeuronCores. Returns (full_output, BassKernelResults)."""
    in_maps = _prep_in_maps(**inputs)
    nc = _get_nc()
    res = run_bass_kernel_spmd(
        nc, in_maps, core_ids=list(range(NCORES)), trace=trace
    )
    out = np.stack([res.results[i]["out"] for i in range(NCORES)])
    return out.reshape(B, C, H, W).astype(np.float32, copy=False), res


def kernel(x, w1, w2, wA, wB):
    out, _ = run({"x": x, "w1": w1, "w2": w2, "wA": wA, "wB": wB})
    return out
